# revision 15
# baseline (speedup 1.0000x reference)
"""CCPL contrastive-loss kernel for Trainium2 (8 NeuronCores).

Strategy: the loss only touches 256 sampled 3x3 neighborhoods of
feat_q/feat_k (~4.7 MB of each 512 MiB tensor), so the kernel never
streams the full tensors.  Work is data-parallel over the batch dim:
core b receives feat_q[b] / feat_k[b] re-laid-out channel-last
([H*W, 128] with q on channels 0-63, k on 64-127), so each sampled
pixel's 128 channels are one contiguous 512 B run in HBM.  The 2304
window positions (256 samples x 9) are gathered by two
indirect_dma_start instructions (SWDGE descriptor generation:
~1 us fixed + 0.34 ns/descriptor) using a [128, 18] int32 row-index
table that ships as *data*, so the program never recompiles when
sample_ids change.  Samples land on SBUF partitions (s%128), channels
on the free axis; per-(sample, position, tensor) L2 norms are free-axis
block reductions on DVE, the normalize/subtract pipeline is split
across Pool/ACT/DVE, and the final cross-partition sum is one PE
matmul.  The host sums the 8 per-core partials and divides by the
element count.
"""

import os
import sys
from contextlib import ExitStack

import numpy as np

sys.path.insert(0, "/opt/trn_rl_repo")

import concourse.bass as bass
import concourse.tile as tile
from concourse import mybir
from concourse.bass_utils import run_bass_kernel_spmd


def _install_ntff_hook():
    """Provide antenv.axon_hooks when the agent image lacks it.

    concourse's axon trace path imports antenv.axon_hooks to fetch the
    NTFF profile hook; this image's antenv has no such submodule.  The
    hook implementation ships in trn_agent_boot.trn_boot, so wire it up
    against the axon PJRT .so directly.
    """
    try:
        from antenv.axon_hooks import get_axon_ntff_profile_hook  # noqa: F401

        return
    except ImportError:
        pass
    import types

    hook = None
    try:
        from trn_agent_boot.trn_boot import _ntff_profile_via_ctypes

        so = "/opt/axon/libaxon_pjrt.so"
        if os.path.exists(so):
            hook = _ntff_profile_via_ctypes(so)
    except Exception:
        hook = None
    mod = types.ModuleType("antenv.axon_hooks")
    _state = {"hook": hook}
    mod.get_axon_ntff_profile_hook = lambda: _state["hook"]
    mod.set_axon_ntff_profile_hook = lambda h: _state.update(hook=h)
    import antenv

    sys.modules["antenv.axon_hooks"] = mod
    antenv.axon_hooks = mod


_install_ntff_hook()

B, C, H, W = 8, 64, 512, 512
NUM_S = 256
P = 2 * C  # q on channels 0-63, k on 64-127 of the channel-last layout
NSLOT = 2  # 256 samples -> 2 slots of 128 (sample s = slot*128 + partition)
N_CORES = 8

_cache: dict = {}
LAST_RESULTS = None  # BassKernelResults of the most recent run (for test.py)


def _split_multi_waits(nc):
    """Walrus build here embeds at most ONE sync wait per instruction.

    Tile emits instructions (notably the kernel-tail Drain) carrying many
    sem waits.  Hoist all but the last wait of any such instruction onto
    single-wait NOPs inserted immediately before it on the same queue —
    the queue stalls on each NOP in turn, preserving semantics.
    """
    from concourse import mybir as _mybir

    for f in nc.m.functions:
        for blk in f.blocks:
            insts = blk.instructions
            i = 0
            while i < len(insts):
                inst = insts[i]
                si = inst.sync_info
                if si is not None and si.on_wait and len(si.on_wait) > 1:
                    waits = list(si.on_wait)
                    si.on_wait = waits[-1:]
                    for j, w in enumerate(waits[:-1]):
                        nop = _mybir.InstNoOp(
                            name=nc.get_next_instruction_name(),
                            ins=[],
                            outs=[],
                            engine=inst.engine,
                            sync_info=_mybir.SyncInfo(on_wait=[w], on_update=[]),
                        )
                        insts.insert(i + j, nop)
                    i += len(waits) - 1
                i += 1


def _build(split_waits=True):
    f32 = mybir.dt.float32
    i32 = mybir.dt.int32
    sub = mybir.AluOpType.subtract
    mul = mybir.AluOpType.mult
    add = mybir.AluOpType.add
    nc = bass.Bass()

    # window-contiguous layout: entry e = h*512 + w holds the 3x128
    # channel-last values of rows h..h+2 at column w, so a full 3x3
    # window is 3 consecutive entries (4608 B).  The HW indirect-DMA
    # ucode consumes exactly ONE offset per destination partition
    # (multi-offset APs silently misgather), so each gather call brings
    # one whole window per partition: 2 calls cover all 256 samples.
    fqk3 = nc.dram_tensor("fqk3", [(H - 2) * W, 3 * P], f32, kind="ExternalInput")
    # idx[p, t] = h*512 + w for sample s = t*128 + p
    idxT = nc.dram_tensor("idx", [128, NSLOT], i32, kind="ExternalInput")
    out = nc.dram_tensor("out", [NSLOT, 1], f32, kind="ExternalOutput")

    with tile.TileContext(nc) as tc, ExitStack() as ctx:
        sb = ctx.enter_context(tc.tile_pool(name="sb", bufs=1))
        pf = ctx.enter_context(tc.tile_pool(name="pf", bufs=1, space="PSUM"))

        ones = sb.tile([128, 1], f32)
        nc.vector.memset(ones[:], 1.0)
        idx = sb.tile([128, NSLOT], i32)
        # ACT's HWDGE reaches this DMA ~1.5us earlier than Sync's queue
        # (shorter engine preamble), and the gathers gate on it.
        nc.scalar.dma_start(out=idx[:], in_=idxT[:])
        # engine warmups: PE clock + ACT sqrt-table load happen off the
        # critical path while the index table streams in.
        warm = pf.tile([1, 1], f32, tag="warm")
        nc.tensor.matmul(out=warm[:], lhsT=ones[:], rhs=ones[:], start=True, stop=True)
        actw = sb.tile([128, 1], f32)
        nc.scalar.sqrt(out=actw[:], in_=ones[:])
        tiny = sb.tile([128, 1], f32)
        nc.vector.memset(tiny[:], 1e-14)

        qk = sb.tile([128, NSLOT, 9, P], f32)  # gathered windows
        d = sb.tile([128, NSLOT, 9, P], f32)  # window - center
        d2 = sb.tile([128, NSLOT, 9, P], f32)
        xh = sb.tile([128, NSLOT, 9, P], f32)  # normalized (q_hat | k_hat)
        n2 = sb.tile([128, NSLOT, 18], f32)  # block B = j*2 + (0:q, 1:k)
        nrm = sb.tile([128, NSLOT, 18], f32)
        rinv = sb.tile([128, NSLOT, 18], f32)
        dif = sb.tile([128, NSLOT, 9, C], f32)
        difa = sb.tile([128, NSLOT, 9 * C], f32)
        acc = sb.tile([128, NSLOT], f32)

        # one SWDGE gather per slot: 128 descriptors x 4608 B (whole window)
        for t in range(NSLOT):
            nc.gpsimd.indirect_dma_start(
                out=qk[:, t],
                out_offset=None,
                in_=fqk3[:],
                in_offset=bass.IndirectOffsetOnAxis(ap=idx[:, t : t + 1], axis=0),
            )

        for t in range(NSLOT):
            # center-subtract: slot 0 on DVE (Pool has just finished
            # descriptor generation), slot 1 on Pool (DVE is mid-pipeline
            # and Pool is otherwise idle).
            ctr = qk[:, t, 4:5, :].to_broadcast([128, 9, P])
            eng = nc.vector if t == 0 else nc.gpsimd
            eng.tensor_tensor(out=d[:, t], in0=qk[:, t], in1=ctr, op=sub)
            nc.scalar.square(out=d2[:, t], in_=d[:, t])
            # norm^2 per (position, tensor) block of 64 channels
            d2b = d2[:, t].rearrange("p j (b c) -> p (j b) c", b=2)
            nc.vector.tensor_reduce(
                out=n2[:, t], in_=d2b, axis=mybir.AxisListType.X, op=add
            )
            # rinv = 1/sqrt(norm2 + tiny); center block norm2=0 -> d=0 -> 0
            nc.scalar.activation(
                out=nrm[:, t], in_=n2[:, t],
                func=mybir.ActivationFunctionType.Sqrt, bias=tiny[:],
            )
            nc.vector.reciprocal(out=rinv[:, t], in_=nrm[:, t])
            # normalize both halves in one DVE pass over the 18 blocks
            db = d[:, t].rearrange("p j (b c) -> p (j b) c", b=2)
            xb = xh[:, t].rearrange("p j (b c) -> p (j b) c", b=2)
            nc.vector.tensor_tensor(
                out=xb,
                in0=db,
                in1=rinv[:, t].unsqueeze(2).to_broadcast([128, 18, C]),
                op=mul,
            )
            xq = xh[:, t].rearrange("p j (b c) -> p j b c", b=2)
            # q_hat - k_hat on Pool (it is idle after the gathers)
            nc.gpsimd.tensor_tensor(
                out=dif[:, t], in0=xq[:, :, 0], in1=xq[:, :, 1], op=sub
            )
            # |dif| summed per partition on ACT (fused abs + accumulate)
            diff = dif[:, t].rearrange("p j c -> p (j c)")
            nc.scalar.activation(
                out=difa[:, t],
                in_=diff,
                func=mybir.ActivationFunctionType.Abs,
                accum_out=acc[:, t : t + 1],
            )

        # cross-partition sum: out[t] = sum_p acc[p, t]
        pfin = pf.tile([NSLOT, 1], f32, tag="fin")
        nc.tensor.matmul(out=pfin[:], lhsT=acc[:], rhs=ones[:], start=True, stop=True)
        res = sb.tile([NSLOT, 1], f32)
        nc.scalar.copy(out=res[:], in_=pfin[:])
        nc.sync.dma_start(out=out[:], in_=res[:])

    if split_waits:
        _split_multi_waits(nc)
    return nc


def kernel(feat_q, feat_k, sample_ids, *, trace=False, trace_cores=None):
    global LAST_RESULTS
    feat_q = np.asarray(feat_q, dtype=np.float32)
    feat_k = np.asarray(feat_k, dtype=np.float32)
    ids = np.asarray(sample_ids).astype(np.int64)

    if "prog" not in _cache:
        _cache["prog"] = _build()
    nc = _cache["prog"]

    # idx[p, t] = flat window-entry index (h*512 + w) for sample t*128 + p
    pos = (ids[:, 0] * W + ids[:, 1]).reshape(NSLOT, 128).T  # [128, NSLOT]
    idx = np.ascontiguousarray(pos).astype(np.int32)

    in_maps = []
    for b in range(N_CORES):
        fqk = np.concatenate([feat_q[b], feat_k[b]], axis=0)  # [128, H, W]
        fqkT = np.ascontiguousarray(fqk.transpose(1, 2, 0))  # [H, W, 128]
        # entry (h, w) = rows h..h+2 at column w: [3, 128] channel-last
        fqk3 = np.empty((H - 2, W, 3, P), dtype=np.float32)
        for r in range(3):
            fqk3[:, :, r, :] = fqkT[r : r + H - 2]
        in_maps.append({"fqk3": fqk3.reshape((H - 2) * W, 3 * P), "idx": idx})

    results = run_bass_kernel_spmd(
        nc,
        in_maps,
        core_ids=list(range(N_CORES)),
        trace=trace,
        trace_cores=trace_cores,
    )
    LAST_RESULTS = results
    total = np.float64(0.0)
    for r in results.results:
        total += np.float64(r["out"].sum())
    loss = total / (B * C * 8 * NUM_S)
    return np.asarray(loss, dtype=np.float32)


# revision 16
# speedup vs baseline: 2.3598x; 2.3598x over previous
"""CCPL contrastive-loss kernel for Trainium2 (8 NeuronCores).

Strategy: the loss only touches 256 sampled 3x3 neighborhoods of
feat_q/feat_k (~4.7 MB of each 512 MiB tensor), so the kernel never
streams the full tensors.  Work is data-parallel over the batch dim:
core b receives feat_q[b] / feat_k[b] re-laid-out channel-last in bf16
([H*W, 128] with q on channels 0-63, k on 64-127), so each sampled
pixel's 128 channels are one contiguous 256 B run in HBM and window
rows (3 pixels) are 768 B runs.  The gather runs on the SWDGE indirect
path with one offset per destination partition (the HW ucode's
contract): 6 calls of offsets=[128,1] -> out=[128, 768 B], one per
(sample-slot, window-row).  Samples land on SBUF partitions, channels
on the free axis; per-(sample, position, tensor) L2 norms are free-axis
block reductions on DVE (bf16 in, f32 out), the normalize pipeline is
split across DVE/ACT/Pool, |q_hat - k_hat| is summed by ACT's fused
Abs+accumulate, and the final cross-partition sum is one PE matmul.
The sample indices ship as data (int32 [128, 6]), so the program never
recompiles when sample_ids change.  The host sums the 8 per-core
partials and divides by the element count.
"""

import os
import sys
from contextlib import ExitStack

import numpy as np

sys.path.insert(0, "/opt/trn_rl_repo")

import ml_dtypes

import concourse.bass as bass
import concourse.tile as tile
from concourse import mybir
from concourse.bass_utils import run_bass_kernel_spmd


def _install_ntff_hook():
    """Provide antenv.axon_hooks when the agent image lacks it.

    concourse's axon trace path imports antenv.axon_hooks to fetch the
    NTFF profile hook; this image's antenv has no such submodule.  The
    hook implementation ships in trn_agent_boot.trn_boot, so wire it up
    against the axon PJRT .so directly.
    """
    try:
        from antenv.axon_hooks import get_axon_ntff_profile_hook  # noqa: F401

        return
    except ImportError:
        pass
    import types

    hook = None
    try:
        from trn_agent_boot.trn_boot import _ntff_profile_via_ctypes

        so = "/opt/axon/libaxon_pjrt.so"
        if os.path.exists(so):
            hook = _ntff_profile_via_ctypes(so)
    except Exception:
        hook = None
    mod = types.ModuleType("antenv.axon_hooks")
    _state = {"hook": hook}
    mod.get_axon_ntff_profile_hook = lambda: _state["hook"]
    mod.set_axon_ntff_profile_hook = lambda h: _state.update(hook=h)
    import antenv

    sys.modules["antenv.axon_hooks"] = mod
    antenv.axon_hooks = mod


_install_ntff_hook()

B, C, H, W = 8, 64, 512, 512
NUM_S = 256
P = 2 * C  # q on channels 0-63, k on 64-127 of the channel-last layout
NSLOT = 2  # 256 samples -> 2 slots of 128 (sample s = slot*128 + partition)
N_CORES = 8

_cache: dict = {}
LAST_RESULTS = None  # BassKernelResults of the most recent run (for test.py)


def _split_multi_waits(nc):
    """Walrus build here embeds at most ONE sync wait per instruction.

    Tile emits instructions (notably the kernel-tail Drain) carrying many
    sem waits.  Hoist all but the last wait of any such instruction onto
    single-wait NOPs inserted immediately before it on the same queue —
    the queue stalls on each NOP in turn, preserving semantics.
    """
    from concourse import mybir as _mybir

    for f in nc.m.functions:
        for blk in f.blocks:
            insts = blk.instructions
            i = 0
            while i < len(insts):
                inst = insts[i]
                si = inst.sync_info
                if si is not None and si.on_wait and len(si.on_wait) > 1:
                    waits = list(si.on_wait)
                    si.on_wait = waits[-1:]
                    for j, w in enumerate(waits[:-1]):
                        nop = _mybir.InstNoOp(
                            name=nc.get_next_instruction_name(),
                            ins=[],
                            outs=[],
                            engine=inst.engine,
                            sync_info=_mybir.SyncInfo(on_wait=[w], on_update=[]),
                        )
                        insts.insert(i + j, nop)
                    i += len(waits) - 1
                i += 1


def _build(split_waits=True):
    f32 = mybir.dt.float32
    bf16 = mybir.dt.bfloat16
    i32 = mybir.dt.int32
    sub = mybir.AluOpType.subtract
    mul = mybir.AluOpType.mult
    add = mybir.AluOpType.add
    nc = bass.Bass()

    # channel-last stacked features in bf16: row (h*512 + w) holds the
    # 128 q|k channels of pixel (h, w) as one contiguous 256 B run.
    fqkT = nc.dram_tensor("fqkT", [H * W, P], bf16, kind="ExternalInput")
    # idx[p, t*3 + r] = (h+r)*512 + w for sample s = t*128 + p.  The HW
    # indirect-DMA ucode consumes exactly ONE offset per destination
    # partition (multi-offset APs silently misgather), so the window
    # gather is 6 calls of shape offsets=[128,1] -> out=[128, 3*128]
    # (rows w..w+2 are contiguous channel-last, 768 B per partition).
    idxT = nc.dram_tensor("idx", [128, NSLOT * 3], i32, kind="ExternalInput")
    out = nc.dram_tensor("out", [NSLOT, 1], f32, kind="ExternalOutput")

    with tile.TileContext(nc) as tc, ExitStack() as ctx:
        sb = ctx.enter_context(tc.tile_pool(name="sb", bufs=1))
        pf = ctx.enter_context(tc.tile_pool(name="pf", bufs=1, space="PSUM"))

        idx = sb.tile([128, NSLOT * 3], i32)
        # load the index table via the Pool queue itself: Pool's preamble
        # clears earliest, and the gathers queue right behind it.
        nc.gpsimd.dma_start(out=idx[:], in_=idxT[:])

        ones = sb.tile([128, 1], f32)
        nc.vector.memset(ones[:], 1.0)
        # engine warmups: PE clock + ACT sqrt-table load happen off the
        # critical path while the index table streams in.
        warm = pf.tile([1, 1], f32, tag="warm")
        nc.tensor.matmul(out=warm[:], lhsT=ones[:], rhs=ones[:], start=True, stop=True)
        actw = sb.tile([128, 1], f32)
        nc.scalar.sqrt(out=actw[:], in_=ones[:])
        tiny = sb.tile([128, 1], f32)
        nc.vector.memset(tiny[:], 1e-14)

        qk = sb.tile([128, NSLOT, 9, P], bf16)  # gathered windows
        d = sb.tile([128, NSLOT, 9, P], bf16)  # window - center
        d2 = sb.tile([128, NSLOT, 9, P], bf16)
        xh = sb.tile([128, NSLOT, 9, P], bf16)  # normalized (q_hat | k_hat)
        n2 = sb.tile([128, NSLOT, 18], f32)  # block B = j*2 + (0:q, 1:k)
        nrm = sb.tile([128, NSLOT, 18], f32)
        rinv = sb.tile([128, NSLOT, 18], f32)
        dif = sb.tile([128, NSLOT, 9, C], bf16)
        difa = sb.tile([128, NSLOT, 9 * C], bf16)
        acc = sb.tile([128, NSLOT], f32)

        # 6 SWDGE gathers (slot-major so slot 0 lands first): each brings
        # one window row (3 positions x 128 ch, 768 B) for 128 samples.
        qkr = qk[:].rearrange("p t (r dw) c -> p t r (dw c)", r=3)
        for t in range(NSLOT):
            for r in range(3):
                nc.gpsimd.indirect_dma_start(
                    out=qkr[:, t, r],
                    out_offset=None,
                    in_=fqkT[:],
                    in_offset=bass.IndirectOffsetOnAxis(
                        ap=idx[:, t * 3 + r : t * 3 + r + 1], axis=0
                    ),
                )

        for t in range(NSLOT):
            # center-subtract: slot 0 on DVE (Pool has just finished
            # descriptor generation), slot 1 on Pool (DVE is mid-pipeline
            # and Pool is otherwise idle).
            ctr = qk[:, t, 4:5, :].to_broadcast([128, 9, P])
            eng = nc.vector if t == 0 else nc.gpsimd
            eng.tensor_tensor(out=d[:, t], in0=qk[:, t], in1=ctr, op=sub)
            nc.scalar.square(out=d2[:, t], in_=d[:, t])
            # norm^2 per (position, tensor) block of 64 channels
            d2b = d2[:, t].rearrange("p j (b c) -> p (j b) c", b=2)
            nc.vector.tensor_reduce(
                out=n2[:, t], in_=d2b, axis=mybir.AxisListType.X, op=add
            )
            # rinv = 1/sqrt(norm2 + tiny); center block norm2=0 -> d=0 -> 0
            nc.scalar.activation(
                out=nrm[:, t], in_=n2[:, t],
                func=mybir.ActivationFunctionType.Sqrt, bias=tiny[:],
            )
            nc.vector.reciprocal(out=rinv[:, t], in_=nrm[:, t])
            # normalize both halves in one DVE pass over the 18 blocks
            db = d[:, t].rearrange("p j (b c) -> p (j b) c", b=2)
            xb = xh[:, t].rearrange("p j (b c) -> p (j b) c", b=2)
            nc.vector.tensor_tensor(
                out=xb,
                in0=db,
                in1=rinv[:, t].unsqueeze(2).to_broadcast([128, 18, C]),
                op=mul,
            )
            xq = xh[:, t].rearrange("p j (b c) -> p j b c", b=2)
            # q_hat - k_hat on Pool (it is idle after the gathers)
            nc.gpsimd.tensor_tensor(
                out=dif[:, t], in0=xq[:, :, 0], in1=xq[:, :, 1], op=sub
            )
            # |dif| summed per partition on ACT (fused abs + accumulate)
            diff = dif[:, t].rearrange("p j c -> p (j c)")
            nc.scalar.activation(
                out=difa[:, t],
                in_=diff,
                func=mybir.ActivationFunctionType.Abs,
                accum_out=acc[:, t : t + 1],
            )

        # cross-partition sum: out[t] = sum_p acc[p, t]
        pfin = pf.tile([NSLOT, 1], f32, tag="fin")
        nc.tensor.matmul(out=pfin[:], lhsT=acc[:], rhs=ones[:], start=True, stop=True)
        res = sb.tile([NSLOT, 1], f32)
        nc.scalar.copy(out=res[:], in_=pfin[:])
        nc.sync.dma_start(out=out[:], in_=res[:])

    if split_waits:
        _split_multi_waits(nc)
    return nc


def kernel(feat_q, feat_k, sample_ids, *, trace=False, trace_cores=None):
    global LAST_RESULTS
    feat_q = np.asarray(feat_q, dtype=np.float32)
    feat_k = np.asarray(feat_k, dtype=np.float32)
    ids = np.asarray(sample_ids).astype(np.int64)

    if "prog" not in _cache:
        _cache["prog"] = _build()
    nc = _cache["prog"]

    # idx[p, t*3 + r] = flat position of window row r for sample t*128 + p
    hs, ws = ids[:, 0], ids[:, 1]
    r = np.arange(3)
    rowpos = (hs[:, None] + r[None, :]) * W + ws[:, None]  # [256, 3]
    idx = np.ascontiguousarray(
        rowpos.reshape(NSLOT, 128, 3).transpose(1, 0, 2).reshape(128, NSLOT * 3)
    ).astype(np.int32)

    in_maps = []
    for b in range(N_CORES):
        fqk = np.concatenate([feat_q[b], feat_k[b]], axis=0)  # [128, H, W]
        fqkT = np.ascontiguousarray(fqk.transpose(1, 2, 0)).reshape(H * W, P)
        in_maps.append({"fqkT": fqkT.astype(ml_dtypes.bfloat16), "idx": idx})

    results = run_bass_kernel_spmd(
        nc,
        in_maps,
        core_ids=list(range(N_CORES)),
        trace=trace,
        trace_cores=trace_cores,
    )
    LAST_RESULTS = results
    total = np.float64(0.0)
    for r_ in results.results:
        total += np.float64(r_["out"].sum())
    loss = total / (B * C * 8 * NUM_S)
    return np.asarray(loss, dtype=np.float32)


# revision 23
# speedup vs baseline: 2.5561x; 1.0832x over previous
"""CCPL contrastive-loss kernel for Trainium2 (8 NeuronCores).

Strategy: the loss only touches 256 sampled 3x3 neighborhoods of
feat_q/feat_k (~4.7 MB of each 512 MiB tensor), so the kernel never
streams the full tensors.  Work is data-parallel over the batch dim:
core b receives feat_q[b] / feat_k[b] re-laid-out channel-last in bf16
([H*W, 128] with q on channels 0-63, k on 64-127), so each sampled
pixel's 128 channels are one contiguous 256 B run in HBM and window
rows (3 pixels) are 768 B runs.  The gather runs on the SWDGE indirect
path with one offset per destination partition (the HW ucode's
contract): 6 calls of offsets=[128,1] -> out=[128, 768 B], one per
(sample-slot, window-row).  Samples land on SBUF partitions, channels
on the free axis; per-(sample, position, tensor) L2 norms are free-axis
block reductions on DVE (bf16 in, f32 out), the normalize pipeline is
split across DVE/ACT/Pool, |q_hat - k_hat| is summed by ACT's fused
Abs+accumulate, and the final cross-partition sum is one PE matmul.
The sample indices ship as data (int32 [128, 6]), so the program never
recompiles when sample_ids change.  The host sums the 8 per-core
partials and divides by the element count.
"""

import os
import sys
from contextlib import ExitStack

import numpy as np

sys.path.insert(0, "/opt/trn_rl_repo")

import ml_dtypes

import concourse.bass as bass
import concourse.tile as tile
from concourse import mybir
from concourse.bass_utils import run_bass_kernel_spmd


def _install_ntff_hook():
    """Provide antenv.axon_hooks when the agent image lacks it.

    concourse's axon trace path imports antenv.axon_hooks to fetch the
    NTFF profile hook; this image's antenv has no such submodule.  The
    hook implementation ships in trn_agent_boot.trn_boot, so wire it up
    against the axon PJRT .so directly.
    """
    try:
        from antenv.axon_hooks import get_axon_ntff_profile_hook  # noqa: F401

        return
    except ImportError:
        pass
    import types

    hook = None
    try:
        from trn_agent_boot.trn_boot import _ntff_profile_via_ctypes

        so = "/opt/axon/libaxon_pjrt.so"
        if os.path.exists(so):
            hook = _ntff_profile_via_ctypes(so)
    except Exception:
        hook = None
    mod = types.ModuleType("antenv.axon_hooks")
    _state = {"hook": hook}
    mod.get_axon_ntff_profile_hook = lambda: _state["hook"]
    mod.set_axon_ntff_profile_hook = lambda h: _state.update(hook=h)
    import antenv

    sys.modules["antenv.axon_hooks"] = mod
    antenv.axon_hooks = mod


_install_ntff_hook()

B, C, H, W = 8, 64, 512, 512
NUM_S = 256
P = 2 * C  # q on channels 0-63, k on 64-127 of the channel-last layout
NSLOT = 2  # 256 samples -> 2 slots of 128 (sample s = slot*128 + partition)
N_CORES = 8

_cache: dict = {}
LAST_RESULTS = None  # BassKernelResults of the most recent run (for test.py)


def _split_multi_waits(nc):
    """Walrus build here embeds at most ONE sync wait per instruction.

    Tile emits instructions (notably the kernel-tail Drain) carrying many
    sem waits.  Hoist all but the last wait of any such instruction onto
    single-wait NOPs inserted immediately before it on the same queue —
    the queue stalls on each NOP in turn, preserving semantics.
    """
    from concourse import mybir as _mybir

    for f in nc.m.functions:
        for blk in f.blocks:
            insts = blk.instructions
            i = 0
            while i < len(insts):
                inst = insts[i]
                si = inst.sync_info
                if si is not None and si.on_wait and len(si.on_wait) > 1:
                    waits = list(si.on_wait)
                    si.on_wait = waits[-1:]
                    for j, w in enumerate(waits[:-1]):
                        nop = _mybir.InstNoOp(
                            name=nc.get_next_instruction_name(),
                            ins=[],
                            outs=[],
                            engine=inst.engine,
                            sync_info=_mybir.SyncInfo(on_wait=[w], on_update=[]),
                        )
                        insts.insert(i + j, nop)
                    i += len(waits) - 1
                i += 1


def _build(split_waits=True):
    f32 = mybir.dt.float32
    bf16 = mybir.dt.bfloat16
    i32 = mybir.dt.int32
    sub = mybir.AluOpType.subtract
    mul = mybir.AluOpType.mult
    add = mybir.AluOpType.add
    nc = bass.Bass()

    # channel-last stacked features in bf16: row (h*512 + w) holds the
    # 128 q|k channels of pixel (h, w) as one contiguous 256 B run.
    fqkT = nc.dram_tensor("fqkT", [H * W, P], bf16, kind="ExternalInput")
    # idx[p, t*3 + r] = (h+r)*512 + w for sample s = t*128 + p.  The HW
    # indirect-DMA ucode consumes exactly ONE offset per destination
    # partition (multi-offset APs silently misgather), so the window
    # gather is 6 calls of shape offsets=[128,1] -> out=[128, 3*128]
    # (rows w..w+2 are contiguous channel-last, 768 B per partition).
    idxT = nc.dram_tensor("idx", [128, NSLOT * 3], i32, kind="ExternalInput")
    out = nc.dram_tensor("out", [NSLOT, 1], f32, kind="ExternalOutput")

    with tile.TileContext(nc) as tc, ExitStack() as ctx:
        sb = ctx.enter_context(tc.tile_pool(name="sb", bufs=1))
        pf = ctx.enter_context(tc.tile_pool(name="pf", bufs=1, space="PSUM"))

        idx = sb.tile([128, NSLOT * 3], i32)
        nc.sync.dma_start(out=idx[:], in_=idxT[:])

        ones = sb.tile([128, 1], f32)
        nc.vector.memset(ones[:], 1.0)
        # engine warmups: PE clock + ACT sqrt-table load happen off the
        # critical path while the index table streams in.
        warm = pf.tile([1, 1], f32, tag="warm")
        nc.tensor.matmul(out=warm[:], lhsT=ones[:], rhs=ones[:], start=True, stop=True)
        actw = sb.tile([128, 1], f32)
        nc.scalar.sqrt(out=actw[:], in_=ones[:])
        tiny = sb.tile([128, 1], f32)
        nc.vector.memset(tiny[:], 1e-14)

        qk = sb.tile([128, NSLOT, 9, P], bf16)  # gathered windows
        d = sb.tile([128, NSLOT, 9, P], bf16)  # window - center
        d2 = sb.tile([128, NSLOT, 9, P], bf16)
        xh = sb.tile([128, NSLOT, 9, P], bf16)  # normalized (q_hat | k_hat)
        n2 = sb.tile([128, NSLOT, 18], bf16)  # block B = j*2 + (0:q, 1:k)
        nrm = sb.tile([128, NSLOT, 18], bf16)
        rinv = sb.tile([128, NSLOT, 18], bf16)
        dif = sb.tile([128, NSLOT, 9, C], bf16)
        difa = sb.tile([128, NSLOT, 9 * C], bf16)
        acc = sb.tile([128, NSLOT], f32)

        # 6 SWDGE gathers (slot-major so slot 0 lands first): each brings
        # one window row (3 positions x 128 ch, 768 B) for 128 samples.
        qkr = qk[:].rearrange("p t (r dw) c -> p t r (dw c)", r=3)
        for t in range(NSLOT):
            for r in range(3):
                nc.gpsimd.indirect_dma_start(
                    out=qkr[:, t, r],
                    out_offset=None,
                    in_=fqkT[:],
                    in_offset=bass.IndirectOffsetOnAxis(
                        ap=idx[:, t * 3 + r : t * 3 + r + 1], axis=0
                    ),
                )

        def slot_compute(t, _lp=nc.allow_low_precision):
            lp = ctx.enter_context(
                _lp("bf16 norm pipeline: ~2e-3 rel on per-column norms is "
                    "far inside the 2e-2 loss gate (measured 3e-5 overall)")
            )
            ctr = qk[:, t, 4:5, :].to_broadcast([128, 9, P])
            nc.vector.tensor_tensor(out=d[:, t], in0=qk[:, t], in1=ctr, op=sub)
            # square: slot 0 on ACT (DVE is the bottleneck engine), slot 1
            # on DVE in bf16 (ACT would serialize behind slot 0's tail).
            if t == 0:
                nc.scalar.square(out=d2[:, t], in_=d[:, t])
            else:
                nc.vector.tensor_tensor(
                    out=d2[:, t], in0=d[:, t], in1=d[:, t], op=mul
                )
            # norm^2 per (position, tensor) block of 64 channels
            d2b = d2[:, t].rearrange("p j (b c) -> p (j b) c", b=2)
            nc.vector.tensor_reduce(
                out=n2[:, t], in_=d2b, axis=mybir.AxisListType.X, op=add
            )
            # rinv = 1/sqrt(norm2 + tiny); center block norm2=0 -> d=0 -> 0
            nc.scalar.activation(
                out=nrm[:, t], in_=n2[:, t],
                func=mybir.ActivationFunctionType.Sqrt, bias=tiny[:],
            )
            nc.vector.reciprocal(out=rinv[:, t], in_=nrm[:, t])
            # normalize both halves in one DVE pass over the 18 blocks
            db = d[:, t].rearrange("p j (b c) -> p (j b) c", b=2)
            xb = xh[:, t].rearrange("p j (b c) -> p (j b) c", b=2)
            nc.vector.tensor_tensor(
                out=xb,
                in0=db,
                in1=rinv[:, t].unsqueeze(2).to_broadcast([128, 18, C]),
                op=mul,
            )
            xq = xh[:, t].rearrange("p j (b c) -> p j b c", b=2)
            # q_hat - k_hat on Pool (it is idle after the gathers)
            nc.gpsimd.tensor_tensor(
                out=dif[:, t], in0=xq[:, :, 0], in1=xq[:, :, 1], op=sub
            )
            # |dif| summed per partition on ACT (fused abs + accumulate)
            diff = dif[:, t].rearrange("p j c -> p (j c)")
            nc.scalar.activation(
                out=difa[:, t],
                in_=diff,
                func=mybir.ActivationFunctionType.Abs,
                accum_out=acc[:, t : t + 1],
            )

        # slot 0's chain is scheduled at elevated priority so the list
        # scheduler never hoists slot 1's bulk ops ahead of slot 0's tiny
        # norm ops on ACT (observed +5us critical-path cost).
        with tc.high_priority(offset=64):
            slot_compute(0)
        slot_compute(1)

        # cross-partition sum: out[t] = sum_p acc[p, t]
        pfin = pf.tile([NSLOT, 1], f32, tag="fin")
        nc.tensor.matmul(out=pfin[:], lhsT=acc[:], rhs=ones[:], start=True, stop=True)
        res = sb.tile([NSLOT, 1], f32)
        nc.scalar.copy(out=res[:], in_=pfin[:])
        nc.sync.dma_start(out=out[:], in_=res[:])

    if split_waits:
        _split_multi_waits(nc)
    return nc


def kernel(feat_q, feat_k, sample_ids, *, trace=False, trace_cores=None):
    global LAST_RESULTS
    feat_q = np.asarray(feat_q, dtype=np.float32)
    feat_k = np.asarray(feat_k, dtype=np.float32)
    ids = np.asarray(sample_ids).astype(np.int64)

    if "prog" not in _cache:
        _cache["prog"] = _build()
    nc = _cache["prog"]

    # idx[p, t*3 + r] = flat position of window row r for sample t*128 + p
    hs, ws = ids[:, 0], ids[:, 1]
    r = np.arange(3)
    rowpos = (hs[:, None] + r[None, :]) * W + ws[:, None]  # [256, 3]
    idx = np.ascontiguousarray(
        rowpos.reshape(NSLOT, 128, 3).transpose(1, 0, 2).reshape(128, NSLOT * 3)
    ).astype(np.int32)

    in_maps = []
    for b in range(N_CORES):
        fqk = np.concatenate([feat_q[b], feat_k[b]], axis=0)  # [128, H, W]
        fqkT = np.ascontiguousarray(fqk.transpose(1, 2, 0)).reshape(H * W, P)
        in_maps.append({"fqkT": fqkT.astype(ml_dtypes.bfloat16), "idx": idx})

    results = run_bass_kernel_spmd(
        nc,
        in_maps,
        core_ids=list(range(N_CORES)),
        trace=trace,
        trace_cores=trace_cores,
    )
    LAST_RESULTS = results
    total = np.float64(0.0)
    for r_ in results.results:
        total += np.float64(r_["out"].sum())
    loss = total / (B * C * 8 * NUM_S)
    return np.asarray(loss, dtype=np.float32)


# revision 26
# speedup vs baseline: 2.5780x; 1.0086x over previous
"""CCPL contrastive-loss kernel for Trainium2 (8 NeuronCores).

Strategy: the loss only touches 256 sampled 3x3 neighborhoods of
feat_q/feat_k (~4.7 MB of each 512 MiB tensor), so the kernel never
streams the full tensors.  Work is data-parallel over the batch dim:
core b receives feat_q[b] / feat_k[b] re-laid-out channel-last in bf16
([H*W, 128] with q on channels 0-63, k on 64-127), so each sampled
pixel's 128 channels are one contiguous 256 B run in HBM and window
rows (3 pixels) are 768 B runs.  The gather runs on the SWDGE indirect
path with one offset per destination partition (the HW ucode's
contract): 6 calls of offsets=[128,1] -> out=[128, 768 B], one per
(sample-slot, window-row).  Samples land on SBUF partitions, channels
on the free axis; per-(sample, position, tensor) L2 norms are free-axis
block reductions on DVE (bf16 in, f32 out), the normalize pipeline is
split across DVE/ACT/Pool, |q_hat - k_hat| is summed by ACT's fused
Abs+accumulate, and the final cross-partition sum is one PE matmul.
The sample indices ship as data (int32 [128, 6]), so the program never
recompiles when sample_ids change.  The host sums the 8 per-core
partials and divides by the element count.
"""

import os
import sys
from contextlib import ExitStack

import numpy as np

sys.path.insert(0, "/opt/trn_rl_repo")

import ml_dtypes

import concourse.bass as bass
import concourse.tile as tile
from concourse import mybir
from concourse.bass_utils import run_bass_kernel_spmd


def _install_ntff_hook():
    """Provide antenv.axon_hooks when the agent image lacks it.

    concourse's axon trace path imports antenv.axon_hooks to fetch the
    NTFF profile hook; this image's antenv has no such submodule.  The
    hook implementation ships in trn_agent_boot.trn_boot, so wire it up
    against the axon PJRT .so directly.
    """
    try:
        from antenv.axon_hooks import get_axon_ntff_profile_hook  # noqa: F401

        return
    except ImportError:
        pass
    import types

    hook = None
    try:
        from trn_agent_boot.trn_boot import _ntff_profile_via_ctypes

        so = "/opt/axon/libaxon_pjrt.so"
        if os.path.exists(so):
            hook = _ntff_profile_via_ctypes(so)
    except Exception:
        hook = None
    mod = types.ModuleType("antenv.axon_hooks")
    _state = {"hook": hook}
    mod.get_axon_ntff_profile_hook = lambda: _state["hook"]
    mod.set_axon_ntff_profile_hook = lambda h: _state.update(hook=h)
    import antenv

    sys.modules["antenv.axon_hooks"] = mod
    antenv.axon_hooks = mod


_install_ntff_hook()

B, C, H, W = 8, 64, 512, 512
NUM_S = 256
P = 2 * C  # q on channels 0-63, k on 64-127 of the channel-last layout
NSLOT = 2  # 256 samples -> 2 slots of 128 (sample s = slot*128 + partition)
N_CORES = 8

_cache: dict = {}
LAST_RESULTS = None  # BassKernelResults of the most recent run (for test.py)


def _split_multi_waits(nc):
    """Walrus build here embeds at most ONE sync wait per instruction.

    Tile emits instructions (notably the kernel-tail Drain) carrying many
    sem waits.  Hoist all but the last wait of any such instruction onto
    single-wait NOPs inserted immediately before it on the same queue —
    the queue stalls on each NOP in turn, preserving semantics.
    """
    from concourse import mybir as _mybir

    for f in nc.m.functions:
        for blk in f.blocks:
            insts = blk.instructions
            i = 0
            while i < len(insts):
                inst = insts[i]
                si = inst.sync_info
                if si is not None and si.on_wait and len(si.on_wait) > 1:
                    waits = list(si.on_wait)
                    si.on_wait = waits[-1:]
                    for j, w in enumerate(waits[:-1]):
                        nop = _mybir.InstNoOp(
                            name=nc.get_next_instruction_name(),
                            ins=[],
                            outs=[],
                            engine=inst.engine,
                            sync_info=_mybir.SyncInfo(on_wait=[w], on_update=[]),
                        )
                        insts.insert(i + j, nop)
                    i += len(waits) - 1
                i += 1


def _build(split_waits=True):
    f32 = mybir.dt.float32
    bf16 = mybir.dt.bfloat16
    i32 = mybir.dt.int32
    sub = mybir.AluOpType.subtract
    mul = mybir.AluOpType.mult
    add = mybir.AluOpType.add
    nc = bass.Bass()

    # channel-last stacked features in bf16: row (h*512 + w) holds the
    # 128 q|k channels of pixel (h, w) as one contiguous 256 B run.
    fqkT = nc.dram_tensor("fqkT", [H * W, P], bf16, kind="ExternalInput")
    # idx[p, t*3 + r] = (h+r)*512 + w for sample s = t*128 + p.  The HW
    # indirect-DMA ucode consumes exactly ONE offset per destination
    # partition (multi-offset APs silently misgather), so the window
    # gather is 6 calls of shape offsets=[128,1] -> out=[128, 3*128]
    # (rows w..w+2 are contiguous channel-last, 768 B per partition).
    idxT = nc.dram_tensor("idx", [128, NSLOT * 3], i32, kind="ExternalInput")
    out = nc.dram_tensor("out", [128, NSLOT], f32, kind="ExternalOutput")

    with tile.TileContext(nc) as tc, ExitStack() as ctx:
        sb = ctx.enter_context(tc.tile_pool(name="sb", bufs=1))

        idx = sb.tile([128, NSLOT * 3], i32)
        # column 0 first so gather 0 can launch ~1us before the rest of
        # the table lands; the remaining columns follow in a second DMA.
        nc.sync.dma_start(out=idx[:, 0:1], in_=idxT[:, 0:1])
        nc.sync.dma_start(out=idx[:, 1:], in_=idxT[:, 1:])

        ones = sb.tile([128, 1], f32)
        nc.vector.memset(ones[:], 1.0)
        # ACT sqrt-table load happens off the critical path while the
        # index table streams in.
        actw = sb.tile([128, 1], f32)
        nc.scalar.sqrt(out=actw[:], in_=ones[:])
        tiny = sb.tile([128, 1], f32)
        nc.vector.memset(tiny[:], 1e-14)

        qk = sb.tile([128, NSLOT, 9, P], bf16)  # gathered windows
        d = sb.tile([128, NSLOT, 9, P], bf16)  # window - center
        d2 = sb.tile([128, NSLOT, 9, P], bf16)
        xh = sb.tile([128, NSLOT, 9, P], bf16)  # normalized (q_hat | k_hat)
        n2 = sb.tile([128, NSLOT, 18], bf16)  # block B = j*2 + (0:q, 1:k)
        nrm = sb.tile([128, NSLOT, 18], bf16)
        rinv = sb.tile([128, NSLOT, 18], bf16)
        dif = sb.tile([128, NSLOT, 9, C], bf16)
        difa = sb.tile([128, NSLOT, 9 * C], bf16)
        acc = sb.tile([128, NSLOT], f32)

        # 6 SWDGE gathers (slot-major so slot 0 lands first): each brings
        # one window row (3 positions x 128 ch, 768 B) for 128 samples.
        qkr = qk[:].rearrange("p t (r dw) c -> p t r (dw c)", r=3)
        for t in range(NSLOT):
            for r in range(3):
                nc.gpsimd.indirect_dma_start(
                    out=qkr[:, t, r],
                    out_offset=None,
                    in_=fqkT[:],
                    in_offset=bass.IndirectOffsetOnAxis(
                        ap=idx[:, t * 3 + r : t * 3 + r + 1], axis=0
                    ),
                )

        def slot_compute(t, _lp=nc.allow_low_precision):
            lp = ctx.enter_context(
                _lp("bf16 norm pipeline: ~2e-3 rel on per-column norms is "
                    "far inside the 2e-2 loss gate (measured 4e-4 overall)")
            )
            # center-subtract and square start as soon as window rows 0-1
            # land (position blocks j = r*3+dw, center at j=4 is in row 1);
            # the row-2 parts follow when the third gather completes.
            ctr = qk[:, t, 4:5, :]
            nc.vector.tensor_tensor(
                out=d[:, t, 0:6], in0=qk[:, t, 0:6],
                in1=ctr.to_broadcast([128, 6, P]), op=sub,
            )
            nc.vector.tensor_tensor(
                out=d2[:, t, 0:6], in0=d[:, t, 0:6], in1=d[:, t, 0:6], op=mul
            )
            nc.vector.tensor_tensor(
                out=d[:, t, 6:9], in0=qk[:, t, 6:9],
                in1=ctr.to_broadcast([128, 3, P]), op=sub,
            )
            nc.vector.tensor_tensor(
                out=d2[:, t, 6:9], in0=d[:, t, 6:9], in1=d[:, t, 6:9], op=mul
            )
            # norm^2 per (position, tensor) block of 64 channels
            d2b = d2[:, t].rearrange("p j (b c) -> p (j b) c", b=2)
            nc.vector.tensor_reduce(
                out=n2[:, t], in_=d2b, axis=mybir.AxisListType.X, op=add
            )
            # rinv = 1/sqrt(norm2 + tiny); center block norm2=0 -> d=0 -> 0
            nc.scalar.activation(
                out=nrm[:, t], in_=n2[:, t],
                func=mybir.ActivationFunctionType.Sqrt, bias=tiny[:],
            )
            nc.vector.reciprocal(out=rinv[:, t], in_=nrm[:, t])
            # normalize both halves in one DVE pass over the 18 blocks
            db = d[:, t].rearrange("p j (b c) -> p (j b) c", b=2)
            xb = xh[:, t].rearrange("p j (b c) -> p (j b) c", b=2)
            nc.vector.tensor_tensor(
                out=xb,
                in0=db,
                in1=rinv[:, t].unsqueeze(2).to_broadcast([128, 18, C]),
                op=mul,
            )
            xq = xh[:, t].rearrange("p j (b c) -> p j b c", b=2)
            if t == 0:
                # off the critical path: q_hat - k_hat on Pool (idle after
                # the gathers), |dif| summed on ACT (fused abs+accumulate)
                nc.gpsimd.tensor_tensor(
                    out=dif[:, t], in0=xq[:, :, 0], in1=xq[:, :, 1], op=sub
                )
                nc.scalar.activation(
                    out=difa[:, t],
                    in_=dif[:, t].rearrange("p j c -> p (j c)"),
                    func=mybir.ActivationFunctionType.Abs,
                    accum_out=acc[:, t : t + 1],
                )
            else:
                # critical path: stay on DVE (strided bf16 sub runs 2x;
                # the abs-reduce is DVE-only and faster than ACT's pair)
                nc.vector.tensor_tensor(
                    out=dif[:, t], in0=xq[:, :, 0], in1=xq[:, :, 1], op=sub
                )
                nc.vector.tensor_reduce(
                    out=acc[:, t : t + 1],
                    in_=dif[:, t].rearrange("p j c -> p (j c)"),
                    axis=mybir.AxisListType.X,
                    op=add,
                    apply_absolute_value=True,
                )

        # slot 0's chain is scheduled at elevated priority so the list
        # scheduler never hoists slot 1's bulk ops ahead of slot 0's tiny
        # norm ops on ACT (observed +5us critical-path cost).
        with tc.high_priority(offset=64):
            slot_compute(0)
        slot_compute(1)

        # per-partition partials go straight to DRAM; the host does the
        # final 256-value sum (cheaper than PE-matmul + copy + DMA here)
        nc.sync.dma_start(out=out[:], in_=acc[:])

    if split_waits:
        _split_multi_waits(nc)
    return nc


def kernel(feat_q, feat_k, sample_ids, *, trace=False, trace_cores=None):
    global LAST_RESULTS
    feat_q = np.asarray(feat_q, dtype=np.float32)
    feat_k = np.asarray(feat_k, dtype=np.float32)
    ids = np.asarray(sample_ids).astype(np.int64)

    if "prog" not in _cache:
        _cache["prog"] = _build()
    nc = _cache["prog"]

    # idx[p, t*3 + r] = flat position of window row r for sample t*128 + p
    hs, ws = ids[:, 0], ids[:, 1]
    r = np.arange(3)
    rowpos = (hs[:, None] + r[None, :]) * W + ws[:, None]  # [256, 3]
    idx = np.ascontiguousarray(
        rowpos.reshape(NSLOT, 128, 3).transpose(1, 0, 2).reshape(128, NSLOT * 3)
    ).astype(np.int32)

    in_maps = []
    for b in range(N_CORES):
        fqk = np.concatenate([feat_q[b], feat_k[b]], axis=0)  # [128, H, W]
        fqkT = np.ascontiguousarray(fqk.transpose(1, 2, 0)).reshape(H * W, P)
        in_maps.append({"fqkT": fqkT.astype(ml_dtypes.bfloat16), "idx": idx})

    results = run_bass_kernel_spmd(
        nc,
        in_maps,
        core_ids=list(range(N_CORES)),
        trace=trace,
        trace_cores=trace_cores,
    )
    LAST_RESULTS = results
    total = np.float64(0.0)
    for r_ in results.results:
        total += np.float64(r_["out"].astype(np.float64).sum())
    loss = total / (B * C * 8 * NUM_S)
    return np.asarray(loss, dtype=np.float32)


# revision 30
# speedup vs baseline: 2.6077x; 1.0115x over previous
"""CCPL contrastive-loss kernel for Trainium2 (8 NeuronCores).

Strategy: the loss only touches 256 sampled 3x3 neighborhoods of
feat_q/feat_k (~4.7 MB of each 512 MiB tensor), so the kernel never
streams the full tensors.  Work is data-parallel over the batch dim:
core b receives feat_q[b] / feat_k[b] re-laid-out channel-last in bf16
([H*W, 128] with q on channels 0-63, k on 64-127), so each sampled
pixel's 128 channels are one contiguous 256 B run in HBM and window
rows (3 pixels) are 768 B runs.  The gather runs on the SWDGE indirect
path with one offset per destination partition (the HW ucode's
contract): 6 calls of offsets=[128,1] -> out=[128, 768 B], one per
(sample-slot, window-row).  Samples land on SBUF partitions, channels
on the free axis; per-(sample, position, tensor) L2 norms are free-axis
block reductions on DVE (bf16 in, f32 out), the normalize pipeline is
split across DVE/ACT/Pool, |q_hat - k_hat| is summed by ACT's fused
Abs+accumulate, and the final cross-partition sum is one PE matmul.
The sample indices ship as data (int32 [128, 6]), so the program never
recompiles when sample_ids change.  The host sums the 8 per-core
partials and divides by the element count.
"""

import os
import sys
from contextlib import ExitStack

import numpy as np

sys.path.insert(0, "/opt/trn_rl_repo")

import ml_dtypes

import concourse.bass as bass
import concourse.tile as tile
from concourse import mybir
from concourse.bass_utils import run_bass_kernel_spmd


def _install_ntff_hook():
    """Provide antenv.axon_hooks when the agent image lacks it.

    concourse's axon trace path imports antenv.axon_hooks to fetch the
    NTFF profile hook; this image's antenv has no such submodule.  The
    hook implementation ships in trn_agent_boot.trn_boot, so wire it up
    against the axon PJRT .so directly.
    """
    try:
        from antenv.axon_hooks import get_axon_ntff_profile_hook  # noqa: F401

        return
    except ImportError:
        pass
    import types

    hook = None
    try:
        from trn_agent_boot.trn_boot import _ntff_profile_via_ctypes

        so = "/opt/axon/libaxon_pjrt.so"
        if os.path.exists(so):
            hook = _ntff_profile_via_ctypes(so)
    except Exception:
        hook = None
    mod = types.ModuleType("antenv.axon_hooks")
    _state = {"hook": hook}
    mod.get_axon_ntff_profile_hook = lambda: _state["hook"]
    mod.set_axon_ntff_profile_hook = lambda h: _state.update(hook=h)
    import antenv

    sys.modules["antenv.axon_hooks"] = mod
    antenv.axon_hooks = mod


_install_ntff_hook()

B, C, H, W = 8, 64, 512, 512
NUM_S = 256
P = 2 * C  # q on channels 0-63, k on 64-127 of the channel-last layout
NSLOT = 2  # 256 samples -> 2 slots of 128 (sample s = slot*128 + partition)
N_CORES = 8

_cache: dict = {}
LAST_RESULTS = None  # BassKernelResults of the most recent run (for test.py)


def _split_multi_waits(nc):
    """Walrus build here embeds at most ONE sync wait per instruction.

    Tile emits instructions (notably the kernel-tail Drain) carrying many
    sem waits.  Hoist all but the last wait of any such instruction onto
    single-wait NOPs inserted immediately before it on the same queue —
    the queue stalls on each NOP in turn, preserving semantics.
    """
    from concourse import mybir as _mybir

    for f in nc.m.functions:
        for blk in f.blocks:
            insts = blk.instructions
            i = 0
            while i < len(insts):
                inst = insts[i]
                si = inst.sync_info
                if si is not None and si.on_wait and len(si.on_wait) > 1:
                    waits = list(si.on_wait)
                    si.on_wait = waits[-1:]
                    for j, w in enumerate(waits[:-1]):
                        nop = _mybir.InstNoOp(
                            name=nc.get_next_instruction_name(),
                            ins=[],
                            outs=[],
                            engine=inst.engine,
                            sync_info=_mybir.SyncInfo(on_wait=[w], on_update=[]),
                        )
                        insts.insert(i + j, nop)
                    i += len(waits) - 1
                i += 1


def _build(split_waits=True):
    f32 = mybir.dt.float32
    bf16 = mybir.dt.bfloat16
    i32 = mybir.dt.int32
    sub = mybir.AluOpType.subtract
    mul = mybir.AluOpType.mult
    add = mybir.AluOpType.add
    nc = bass.Bass()

    # channel-last stacked features in bf16: row (h*512 + w) holds the
    # 128 q|k channels of pixel (h, w) as one contiguous 256 B run.
    fqkT = nc.dram_tensor("fqkT", [H * W, P], bf16, kind="ExternalInput")
    # idx[p, t*3 + r] = (h+r)*512 + w for sample s = t*128 + p.  The HW
    # indirect-DMA ucode consumes exactly ONE offset per destination
    # partition (multi-offset APs silently misgather), so the window
    # gather is 6 calls of shape offsets=[128,1] -> out=[128, 3*128]
    # (rows w..w+2 are contiguous channel-last, 768 B per partition).
    idxT = nc.dram_tensor("idx", [128, NSLOT * 3], i32, kind="ExternalInput")
    out = nc.dram_tensor("out", [NSLOT, 1], f32, kind="ExternalOutput")

    with tile.TileContext(nc) as tc, ExitStack() as ctx:
        sb = ctx.enter_context(tc.tile_pool(name="sb", bufs=1))
        pf = ctx.enter_context(tc.tile_pool(name="pf", bufs=1, space="PSUM"))

        idx = sb.tile([128, NSLOT * 3], i32)
        nc.sync.dma_start(out=idx[:], in_=idxT[:])

        ones = sb.tile([128, 1], f32)
        nc.vector.memset(ones[:], 1.0)
        # engine warmups: PE clock + ACT sqrt-table load happen off the
        # critical path while the index table streams in.
        warm = pf.tile([1, 1], f32, tag="warm")
        nc.tensor.matmul(out=warm[:], lhsT=ones[:], rhs=ones[:], start=True, stop=True)
        actw = sb.tile([128, 1], f32)
        nc.scalar.sqrt(out=actw[:], in_=ones[:])
        tiny = sb.tile([128, 1], f32)
        nc.vector.memset(tiny[:], 1e-14)

        qk = sb.tile([128, NSLOT, 9, P], bf16)  # gathered windows
        # slot 1's scratch aliases slot 0's (d <-> d2 swapped): the WAR
        # dependencies force the list scheduler to emit slot 0's reduce
        # and normalize before slot 1's subtract/square on DVE — without
        # them it reorders on its (optimistic) DMA timing model and
        # strands DVE idle behind slot 1's gather.
        dA = sb.tile([128, 9, P], bf16)  # slot0: window-center | slot1: d^2
        dB = sb.tile([128, 9, P], bf16)  # slot0: d^2 | slot1: window-center
        xh = sb.tile([128, NSLOT, 9, P], bf16)  # normalized (q_hat | k_hat)
        n2 = sb.tile([128, NSLOT, 18], bf16)  # block B = j*2 + (0:q, 1:k)
        nrm = sb.tile([128, NSLOT, 18], bf16)
        rinv = sb.tile([128, NSLOT, 18], bf16)
        dif = sb.tile([128, NSLOT, 9, C], bf16)
        difa = sb.tile([128, 9 * C], bf16)
        acc = sb.tile([128, NSLOT], f32)

        # 6 SWDGE gathers (slot-major so slot 0 lands first): each brings
        # one window row (3 positions x 128 ch, 768 B) for 128 samples.
        qkr = qk[:].rearrange("p t (r dw) c -> p t r (dw c)", r=3)
        for t in range(NSLOT):
            for r in range(3):
                nc.gpsimd.indirect_dma_start(
                    out=qkr[:, t, r],
                    out_offset=None,
                    in_=fqkT[:],
                    in_offset=bass.IndirectOffsetOnAxis(
                        ap=idx[:, t * 3 + r : t * 3 + r + 1], axis=0
                    ),
                )

        def slot_compute(t, _lp=nc.allow_low_precision):
            lp = ctx.enter_context(
                _lp("bf16 norm pipeline: ~2e-3 rel on per-column norms is "
                    "far inside the 2e-2 loss gate (measured 4e-4 overall)")
            )
            d = dA if t == 0 else dB
            d2 = dB if t == 0 else dA
            # center-subtract and square start as soon as window rows 0-1
            # land (position blocks j = r*3+dw, center at j=4 is in row 1);
            # the row-2 parts follow when the third gather completes.
            ctr = qk[:, t, 4:5, :]
            nc.vector.tensor_tensor(
                out=d[:, 0:6], in0=qk[:, t, 0:6],
                in1=ctr.to_broadcast([128, 6, P]), op=sub,
            )
            nc.vector.tensor_tensor(
                out=d2[:, 0:6], in0=d[:, 0:6], in1=d[:, 0:6], op=mul
            )
            nc.vector.tensor_tensor(
                out=d[:, 6:9], in0=qk[:, t, 6:9],
                in1=ctr.to_broadcast([128, 3, P]), op=sub,
            )
            nc.vector.tensor_tensor(
                out=d2[:, 6:9], in0=d[:, 6:9], in1=d[:, 6:9], op=mul
            )
            # norm^2 per (position, tensor) block of 64 channels
            d2b = d2[:].rearrange("p j (b c) -> p (j b) c", b=2)
            nc.vector.tensor_reduce(
                out=n2[:, t], in_=d2b, axis=mybir.AxisListType.X, op=add
            )
            # rinv = 1/sqrt(norm2 + tiny); center block norm2=0 -> d=0 -> 0
            nc.scalar.activation(
                out=nrm[:, t], in_=n2[:, t],
                func=mybir.ActivationFunctionType.Sqrt, bias=tiny[:],
            )
            nc.vector.reciprocal(out=rinv[:, t], in_=nrm[:, t])
            # normalize both halves in one DVE pass over the 18 blocks
            db = d[:].rearrange("p j (b c) -> p (j b) c", b=2)
            xb = xh[:, t].rearrange("p j (b c) -> p (j b) c", b=2)
            nc.vector.tensor_tensor(
                out=xb,
                in0=db,
                in1=rinv[:, t].unsqueeze(2).to_broadcast([128, 18, C]),
                op=mul,
            )
            xq = xh[:, t].rearrange("p j (b c) -> p j b c", b=2)
            if t == 0:
                # off the critical path: q_hat - k_hat on Pool (idle after
                # the gathers), |dif| summed on ACT (fused abs+accumulate)
                nc.gpsimd.tensor_tensor(
                    out=dif[:, t], in0=xq[:, :, 0], in1=xq[:, :, 1], op=sub
                )
                nc.scalar.activation(
                    out=difa[:],
                    in_=dif[:, t].rearrange("p j c -> p (j c)"),
                    func=mybir.ActivationFunctionType.Abs,
                    accum_out=acc[:, t : t + 1],
                )
            else:
                # critical path: stay on DVE (strided bf16 sub runs 2x;
                # the abs-reduce is DVE-only and faster than ACT's pair)
                nc.vector.tensor_tensor(
                    out=dif[:, t], in0=xq[:, :, 0], in1=xq[:, :, 1], op=sub
                )
                nc.vector.tensor_reduce(
                    out=acc[:, t : t + 1],
                    in_=dif[:, t].rearrange("p j c -> p (j c)"),
                    axis=mybir.AxisListType.X,
                    op=add,
                    apply_absolute_value=True,
                )

        # slot 0's chain is scheduled at elevated priority so the list
        # scheduler never hoists slot 1's bulk ops ahead of slot 0's tiny
        # norm ops on ACT (observed +5us critical-path cost).
        with tc.high_priority(offset=64):
            slot_compute(0)
        slot_compute(1)

        # cross-partition sum: out[t] = sum_p acc[p, t].  One PE matmul +
        # a single 8 B DRAM write beats DMAing acc[128, 2] (128 scattered
        # 8 B HBM writes each pay a read-modify-write round trip).
        pfin = pf.tile([NSLOT, 1], f32, tag="fin")
        nc.tensor.matmul(out=pfin[:], lhsT=acc[:], rhs=ones[:], start=True, stop=True)
        res = sb.tile([NSLOT, 1], f32)
        nc.scalar.copy(out=res[:], in_=pfin[:])
        nc.sync.dma_start(out=out[:], in_=res[:])

    if split_waits:
        _split_multi_waits(nc)
    return nc


def kernel(feat_q, feat_k, sample_ids, *, trace=False, trace_cores=None):
    global LAST_RESULTS
    feat_q = np.asarray(feat_q, dtype=np.float32)
    feat_k = np.asarray(feat_k, dtype=np.float32)
    ids = np.asarray(sample_ids).astype(np.int64)

    if "prog" not in _cache:
        _cache["prog"] = _build()
    nc = _cache["prog"]

    # idx[p, t*3 + r] = flat position of window row r for sample t*128 + p
    hs, ws = ids[:, 0], ids[:, 1]
    r = np.arange(3)
    rowpos = (hs[:, None] + r[None, :]) * W + ws[:, None]  # [256, 3]
    idx = np.ascontiguousarray(
        rowpos.reshape(NSLOT, 128, 3).transpose(1, 0, 2).reshape(128, NSLOT * 3)
    ).astype(np.int32)

    in_maps = []
    for b in range(N_CORES):
        fqk = np.concatenate([feat_q[b], feat_k[b]], axis=0)  # [128, H, W]
        fqkT = np.ascontiguousarray(fqk.transpose(1, 2, 0)).reshape(H * W, P)
        in_maps.append({"fqkT": fqkT.astype(ml_dtypes.bfloat16), "idx": idx})

    results = run_bass_kernel_spmd(
        nc,
        in_maps,
        core_ids=list(range(N_CORES)),
        trace=trace,
        trace_cores=trace_cores,
    )
    LAST_RESULTS = results
    total = np.float64(0.0)
    for r_ in results.results:
        total += np.float64(r_["out"].astype(np.float64).sum())
    loss = total / (B * C * 8 * NUM_S)
    return np.asarray(loss, dtype=np.float32)


# revision 32
# speedup vs baseline: 2.7316x; 1.0475x over previous
"""CCPL contrastive-loss kernel for Trainium2 (8 NeuronCores).

Strategy: the loss only touches 256 sampled 3x3 neighborhoods of
feat_q/feat_k (~4.7 MB of each 512 MiB tensor), so the kernel never
streams the full tensors.  Work is data-parallel over the batch dim:
core b receives feat_q[b] / feat_k[b] re-laid-out channel-last in bf16
([H*W, 128] with q on channels 0-63, k on 64-127), so each sampled
pixel's 128 channels are one contiguous 256 B run in HBM and window
rows (3 pixels) are 768 B runs.  The gather runs on the SWDGE indirect
path with one offset per destination partition (the HW ucode's
contract): 6 calls of offsets=[128,1] -> out=[128, 768 B], one per
(sample-slot, window-row).  Samples land on SBUF partitions, channels
on the free axis; per-(sample, position, tensor) L2 norms are free-axis
block reductions on DVE (bf16 in, f32 out), the normalize pipeline is
split across DVE/ACT/Pool, |q_hat - k_hat| is summed by ACT's fused
Abs+accumulate, and the final cross-partition sum is one PE matmul.
The sample indices ship as data (int32 [128, 6]), so the program never
recompiles when sample_ids change.  The host sums the 8 per-core
partials and divides by the element count.
"""

import os
import sys
from contextlib import ExitStack

import numpy as np

sys.path.insert(0, "/opt/trn_rl_repo")

import ml_dtypes

import concourse.bass as bass
import concourse.tile as tile
from concourse import mybir
from concourse.bass_utils import run_bass_kernel_spmd


def _install_ntff_hook():
    """Provide antenv.axon_hooks when the agent image lacks it.

    concourse's axon trace path imports antenv.axon_hooks to fetch the
    NTFF profile hook; this image's antenv has no such submodule.  The
    hook implementation ships in trn_agent_boot.trn_boot, so wire it up
    against the axon PJRT .so directly.
    """
    try:
        from antenv.axon_hooks import get_axon_ntff_profile_hook  # noqa: F401

        return
    except ImportError:
        pass
    import types

    hook = None
    try:
        from trn_agent_boot.trn_boot import _ntff_profile_via_ctypes

        so = "/opt/axon/libaxon_pjrt.so"
        if os.path.exists(so):
            hook = _ntff_profile_via_ctypes(so)
    except Exception:
        hook = None
    mod = types.ModuleType("antenv.axon_hooks")
    _state = {"hook": hook}
    mod.get_axon_ntff_profile_hook = lambda: _state["hook"]
    mod.set_axon_ntff_profile_hook = lambda h: _state.update(hook=h)
    import antenv

    sys.modules["antenv.axon_hooks"] = mod
    antenv.axon_hooks = mod


_install_ntff_hook()

B, C, H, W = 8, 64, 512, 512
NUM_S = 256
P = 2 * C  # q on channels 0-63, k on 64-127 of the channel-last layout
NSLOT = 2  # 256 samples -> 2 slots of 128 (sample s = slot*128 + partition)
N_CORES = 8

_cache: dict = {}
LAST_RESULTS = None  # BassKernelResults of the most recent run (for test.py)


def _split_multi_waits(nc):
    """Walrus build here embeds at most ONE sync wait per instruction.

    Tile emits instructions (notably the kernel-tail Drain) carrying many
    sem waits.  Hoist all but the last wait of any such instruction onto
    single-wait NOPs inserted immediately before it on the same queue —
    the queue stalls on each NOP in turn, preserving semantics.
    """
    from concourse import mybir as _mybir

    for f in nc.m.functions:
        for blk in f.blocks:
            insts = blk.instructions
            i = 0
            while i < len(insts):
                inst = insts[i]
                si = inst.sync_info
                if si is not None and si.on_wait and len(si.on_wait) > 1:
                    waits = list(si.on_wait)
                    si.on_wait = waits[-1:]
                    for j, w in enumerate(waits[:-1]):
                        nop = _mybir.InstNoOp(
                            name=nc.get_next_instruction_name(),
                            ins=[],
                            outs=[],
                            engine=inst.engine,
                            sync_info=_mybir.SyncInfo(on_wait=[w], on_update=[]),
                        )
                        insts.insert(i + j, nop)
                    i += len(waits) - 1
                i += 1


def _build(split_waits=True):
    f32 = mybir.dt.float32
    bf16 = mybir.dt.bfloat16
    i32 = mybir.dt.int32
    sub = mybir.AluOpType.subtract
    mul = mybir.AluOpType.mult
    add = mybir.AluOpType.add
    nc = bass.Bass()

    # channel-last stacked features in bf16: row (h*512 + w) holds the
    # 128 q|k channels of pixel (h, w) as one contiguous 256 B run.
    fqkT = nc.dram_tensor("fqkT", [H * W, P], bf16, kind="ExternalInput")
    # idx[p, t*3 + r] = (h+r)*512 + w for sample s = t*128 + p.  The HW
    # indirect-DMA ucode consumes exactly ONE offset per destination
    # partition (multi-offset APs silently misgather), so the window
    # gather is 6 calls of shape offsets=[128,1] -> out=[128, 3*128]
    # (rows w..w+2 are contiguous channel-last, 768 B per partition).
    idxT = nc.dram_tensor("idx", [128, NSLOT * 3], i32, kind="ExternalInput")
    out = nc.dram_tensor("out", [NSLOT, 1], f32, kind="ExternalOutput")

    with tile.TileContext(nc) as tc, ExitStack() as ctx:
        sb = ctx.enter_context(tc.tile_pool(name="sb", bufs=1))
        pf = ctx.enter_context(tc.tile_pool(name="pf", bufs=1, space="PSUM"))

        idx = sb.tile([128, NSLOT * 3], i32)
        nc.sync.dma_start(out=idx[:], in_=idxT[:])

        ones = sb.tile([128, 1], f32)
        nc.vector.memset(ones[:], 1.0)
        # engine warmups: PE clock + ACT sqrt-table load happen off the
        # critical path while the index table streams in.
        warm = pf.tile([1, 1], f32, tag="warm")
        nc.tensor.matmul(out=warm[:], lhsT=ones[:], rhs=ones[:], start=True, stop=True)
        actw = sb.tile([128, 1], f32)
        nc.scalar.sqrt(out=actw[:], in_=ones[:])
        tiny = sb.tile([128, 1], f32)
        nc.vector.memset(tiny[:], 1e-14)

        qk = sb.tile([128, NSLOT, 9, P], bf16)  # gathered windows
        # slot 1's scratch aliases slot 0's (d <-> d2 swapped): the WAR
        # dependencies force the list scheduler to emit slot 0's reduce
        # and normalize before slot 1's subtract/square on DVE — without
        # them it reorders on its (optimistic) DMA timing model and
        # strands DVE idle behind slot 1's gather.
        dA = sb.tile([128, 9, P], bf16)  # slot0: window-center
        dB = sb.tile([128, 9, P], bf16)  # slot0: d^2 | slot1: window-center
        dC = sb.tile([128, 9, P], bf16)  # slot1: d^2
        xh = sb.tile([128, NSLOT, 9, P], bf16)  # normalized (q_hat | k_hat)
        n2 = sb.tile([128, NSLOT, 18], bf16)  # block B = j*2 + (0:q, 1:k)
        nrm = sb.tile([128, NSLOT, 18], bf16)
        rinv = sb.tile([128, NSLOT, 18], bf16)
        dif = sb.tile([128, NSLOT, 9, C], bf16)
        difa = sb.tile([128, 9 * C], bf16)
        acc = sb.tile([128, NSLOT], f32)

        # 6 SWDGE gathers (slot-major so slot 0 lands first): each brings
        # one window row (3 positions x 128 ch, 768 B) for 128 samples.
        qkr = qk[:].rearrange("p t (r dw) c -> p t r (dw c)", r=3)
        for t in range(NSLOT):
            for r in range(3):
                nc.gpsimd.indirect_dma_start(
                    out=qkr[:, t, r],
                    out_offset=None,
                    in_=fqkT[:],
                    in_offset=bass.IndirectOffsetOnAxis(
                        ap=idx[:, t * 3 + r : t * 3 + r + 1], axis=0
                    ),
                )

        def slot_compute(t, _lp=nc.allow_low_precision):
            lp = ctx.enter_context(
                _lp("bf16 norm pipeline: ~2e-3 rel on per-column norms is "
                    "far inside the 2e-2 loss gate (measured 4e-4 overall)")
            )
            d = dA if t == 0 else dB
            d2 = dB if t == 0 else dC
            # center-subtract and square start as soon as window rows 0-1
            # land (position blocks j = r*3+dw, center at j=4 is in row 1);
            # the row-2 parts follow when the third gather completes.
            ctr = qk[:, t, 4:5, :]
            nc.vector.tensor_tensor(
                out=d[:, 0:6], in0=qk[:, t, 0:6],
                in1=ctr.to_broadcast([128, 6, P]), op=sub,
            )
            nc.vector.tensor_tensor(
                out=d2[:, 0:6], in0=d[:, 0:6], in1=d[:, 0:6], op=mul
            )
            nc.vector.tensor_tensor(
                out=d[:, 6:9], in0=qk[:, t, 6:9],
                in1=ctr.to_broadcast([128, 3, P]), op=sub,
            )
            nc.vector.tensor_tensor(
                out=d2[:, 6:9], in0=d[:, 6:9], in1=d[:, 6:9], op=mul
            )
            # norm^2 per (position, tensor) block of 64 channels
            d2b = d2[:].rearrange("p j (b c) -> p (j b) c", b=2)
            nc.vector.tensor_reduce(
                out=n2[:, t], in_=d2b, axis=mybir.AxisListType.X, op=add
            )
            # rinv = 1/sqrt(norm2 + tiny); center block norm2=0 -> d=0 -> 0
            nc.scalar.activation(
                out=nrm[:, t], in_=n2[:, t],
                func=mybir.ActivationFunctionType.Sqrt, bias=tiny[:],
            )
            nc.vector.reciprocal(out=rinv[:, t], in_=nrm[:, t])
            # normalize both halves in one DVE pass over the 18 blocks
            db = d[:].rearrange("p j (b c) -> p (j b) c", b=2)
            xb = xh[:, t].rearrange("p j (b c) -> p (j b) c", b=2)
            nc.vector.tensor_tensor(
                out=xb,
                in0=db,
                in1=rinv[:, t].unsqueeze(2).to_broadcast([128, 18, C]),
                op=mul,
            )
            xq = xh[:, t].rearrange("p j (b c) -> p j b c", b=2)
            if t == 0:
                # off the critical path: q_hat - k_hat on Pool (idle after
                # the gathers), |dif| summed on ACT (fused abs+accumulate)
                nc.gpsimd.tensor_tensor(
                    out=dif[:, t], in0=xq[:, :, 0], in1=xq[:, :, 1], op=sub
                )
                nc.scalar.activation(
                    out=difa[:],
                    in_=dif[:, t].rearrange("p j c -> p (j c)"),
                    func=mybir.ActivationFunctionType.Abs,
                    accum_out=acc[:, t : t + 1],
                )
            else:
                # critical path: stay on DVE (strided bf16 sub runs 2x;
                # the abs-reduce is DVE-only and faster than ACT's pair)
                nc.vector.tensor_tensor(
                    out=dif[:, t], in0=xq[:, :, 0], in1=xq[:, :, 1], op=sub
                )
                nc.vector.tensor_reduce(
                    out=acc[:, t : t + 1],
                    in_=dif[:, t].rearrange("p j c -> p (j c)"),
                    axis=mybir.AxisListType.X,
                    op=add,
                    apply_absolute_value=True,
                )

        # slot 0's chain is scheduled at elevated priority so the list
        # scheduler never hoists slot 1's bulk ops ahead of slot 0's tiny
        # norm ops on ACT (observed +5us critical-path cost).
        with tc.high_priority(offset=64):
            slot_compute(0)
        slot_compute(1)

        # cross-partition sum: out[t] = sum_p acc[p, t].  One PE matmul +
        # a single 8 B DRAM write beats DMAing acc[128, 2] (128 scattered
        # 8 B HBM writes each pay a read-modify-write round trip).
        pfin = pf.tile([NSLOT, 1], f32, tag="fin")
        nc.tensor.matmul(out=pfin[:], lhsT=acc[:], rhs=ones[:], start=True, stop=True)
        res = sb.tile([NSLOT, 1], f32)
        nc.scalar.copy(out=res[:], in_=pfin[:])
        nc.sync.dma_start(out=out[:], in_=res[:])

    if split_waits:
        _split_multi_waits(nc)
    return nc


def kernel(feat_q, feat_k, sample_ids, *, trace=False, trace_cores=None):
    global LAST_RESULTS
    feat_q = np.asarray(feat_q, dtype=np.float32)
    feat_k = np.asarray(feat_k, dtype=np.float32)
    ids = np.asarray(sample_ids).astype(np.int64)

    if "prog" not in _cache:
        _cache["prog"] = _build()
    nc = _cache["prog"]

    # idx[p, t*3 + r] = flat position of window row r for sample t*128 + p
    hs, ws = ids[:, 0], ids[:, 1]
    r = np.arange(3)
    rowpos = (hs[:, None] + r[None, :]) * W + ws[:, None]  # [256, 3]
    idx = np.ascontiguousarray(
        rowpos.reshape(NSLOT, 128, 3).transpose(1, 0, 2).reshape(128, NSLOT * 3)
    ).astype(np.int32)

    in_maps = []
    for b in range(N_CORES):
        fqk = np.concatenate([feat_q[b], feat_k[b]], axis=0)  # [128, H, W]
        fqkT = np.ascontiguousarray(fqk.transpose(1, 2, 0)).reshape(H * W, P)
        in_maps.append({"fqkT": fqkT.astype(ml_dtypes.bfloat16), "idx": idx})

    results = run_bass_kernel_spmd(
        nc,
        in_maps,
        core_ids=list(range(N_CORES)),
        trace=trace,
        trace_cores=trace_cores,
    )
    LAST_RESULTS = results
    total = np.float64(0.0)
    for r_ in results.results:
        total += np.float64(r_["out"].astype(np.float64).sum())
    loss = total / (B * C * 8 * NUM_S)
    return np.asarray(loss, dtype=np.float32)
